# revision 37
# baseline (speedup 1.0000x reference)
"""LoRA-MHSA Trainium2 kernel, v2.

Data-parallel over batch B=8 (one sample per NeuronCore). The per-sample
LoRA adapter is folded into the base weights on the host (exact math:
W_eff = W + (alpha/rank) * B[sid] @ A[sid]), so the device kernel is a
plain MHSA over per-core effective weights.

All matmul operands are bf16 (fp32r streams at ~half rate on real HW;
bf16 streams a column per cycle and enables fast weight load). PSUM
accumulation stays fp32. Tolerance is 2e-2; bf16 lands ~1e-3.

Layout: activations channel-major ([C, T]) so q/k head slabs feed the
scores matmul directly. v is built in natural layout [T, C] with a
65th ones-column per head so the PV matmul emits the softmax denominator
for free in PSUM row 64. Scores for the two heads of a pair occupy
partitions 0-63 / 64-127, so their matmuls row-pack into disjoint PE
row-groups and run concurrently. One exp activation per (pair, tq-chunk,
tk-tile) covers both heads ([128, 1024] across 2 PSUM banks), halving
ACT instruction overhead. Softmax normalization is staged out of PSUM
immediately (frees the PV bank), then reciprocal_approx_fast + gpsimd
partition-broadcast + DVE multiply produce the normalized, bf16,
channel-major attention output off the critical path.

Weights are host-packed into per-slab contiguous blocks so every DMA is
a single contiguous transfer.
"""

import sys
from itertools import chain

sys.path.insert(0, "/opt/trn_rl_repo")

import numpy as np
import ml_dtypes
import concourse.bass as bass
import concourse.tile as tile
from concourse import bacc, mybir
from concourse.bass_utils import run_bass_kernel_spmd

T = 1024
C = 1024
C3 = 3072
H = 16
DH = 64
RANK = 8
AOR = 0.125          # alpha / rank
SM_SCALE = 0.125     # 1/sqrt(dh)
NCORES = 8

F32 = mybir.dt.float32
BF16 = mybir.dt.bfloat16
EXP = mybir.ActivationFunctionType.Exp
BF16NP = ml_dtypes.bfloat16

ts = bass.ts

TT = T // 128     # 8 tk tiles
TCH = T // 512    # 2 tq chunks (psum free dim)
CI = C // 128     # 8 contraction tiles
NPAIR = H // 2    # 8 head pairs
NVC = 4           # v built in 4 chunks of 256 couts (4 heads each)


def _build(dump=False):
    nc = bacc.Bacc("TRN2", target_bir_lowering=False, debug=False)

    # Host-packed layouts (see make_in_maps): every slab is contiguous.
    xTp_d = nc.dram_tensor("xTp", [CI, 128, T], BF16, kind="ExternalInput")
    wqk_d = nc.dram_tensor("wqk", [NPAIR, 128, CI, 256], BF16, kind="ExternalInput")
    wv_d = nc.dram_tensor("wv", [NVC, 128, CI, 256], BF16, kind="ExternalInput")
    wp_d = nc.dram_tensor("wp", [128, CI, C], BF16, kind="ExternalInput")
    out_d = nc.dram_tensor("out", [T, C], F32, kind="ExternalOutput")
    if dump:
        dqkt_d = nc.dram_tensor("dqkt", [128, 2, T], BF16, kind="ExternalOutput")
        dvv_d = nc.dram_tensor("dvv", [128, TT, H, DH + 1], BF16,
                               kind="ExternalOutput")
        des_d = nc.dram_tensor("des", [128, 1024], BF16, kind="ExternalOutput")
        dstg_d = nc.dram_tensor("dstg", [DH + 1, 512], F32, kind="ExternalOutput")
        drb_d = nc.dram_tensor("drb", [DH, 512], F32, kind="ExternalOutput")
        dyt_d = nc.dram_tensor("dyt", [128, CI, T], BF16, kind="ExternalOutput")

    with tile.TileContext(nc) as tc:
      with tc.tile_pool(name="res", bufs=1) as res, \
           tc.tile_pool(name="qkt", bufs=4) as qktp, \
           tc.tile_pool(name="wqk", bufs=2) as wqkp, \
           tc.tile_pool(name="wv", bufs=2) as wvp, \
           tc.tile_pool(name="es", bufs=5) as esp, \
           tc.tile_pool(name="nrm", bufs=2) as nrmp, \
           tc.tile_pool(name="ot", bufs=2) as otp, \
           tc.tile_pool(name="bps", bufs=2, space="PSUM") as bps, \
           tc.tile_pool(name="sps", bufs=2, space="PSUM") as sps, \
           tc.tile_pool(name="yps", bufs=2, space="PSUM") as yps:

        xT = [
            res.tile([128, T], BF16, tag=f"x{ci}", name=f"xT{ci}")
            for ci in range(CI)
        ]
        vv = res.tile([128, TT, H, DH + 1], BF16, tag="vv")
        yt = res.tile([128, CI, T], BF16, tag="yt")
        wpa = res.tile([128, CI, C], BF16, tag="wpa")
        wrm = res.tile([128, 512], BF16, tag="wrm")

        qktiles = {}

        def qk_build(p):
            wqt = wqkp.tile([128, CI, 256], BF16, tag="wq", name="wqt")

            def load(p=p, wqt=wqt):
                if p == 0:
                    # split so the first build matmuls (ci 0-1) unblock early
                    nc.sync.dma_start(out=wqt[:, 0:2, :], in_=wqk_d[p][:, 0:2, :])
                    nc.sync.dma_start(out=wqt[:, 2:CI, :], in_=wqk_d[p][:, 2:CI, :])
                else:
                    nc.sync.dma_start(out=wqt[:], in_=wqk_d[p])

            yield load
            qkt = qktp.tile([128, 2, T], BF16, tag="qkt", name="qkt")
            qktiles[p] = qkt
            for part in range(2):  # 0: q, 1: k
                pqs = [
                    bps.tile([128, 512], F32, tag="pq", name="pq")
                    for _ in range(TCH)
                ]
                for ci in range(CI):
                    def step(ci=ci, part=part, pqs=pqs, wqt=wqt):
                        for tch in range(TCH):
                            nc.tensor.matmul(
                                pqs[tch][:],
                                wqt[:, ci, ts(part, 128)],
                                xT[ci][:, ts(tch, 512)],
                                start=(ci == 0), stop=(ci == CI - 1),
                            )
                    yield step

                def fin(part=part, pqs=pqs, qkt=qkt):
                    for tch in range(TCH):
                        nc.vector.tensor_copy(
                            qkt[:, part, ts(tch, 512)], pqs[tch][:]
                        )
                yield fin

        def v_build(vc):  # heads 4*vc .. 4*vc+3
            wvv = wvp.tile([128, CI, 256], BF16, tag="wv", name="wvv")

            def load(vc=vc, wvv=wvv):
                nc.sync.dma_start(out=wvv[:], in_=wv_d[vc])

            yield load
            for tt in range(TT):
                def step(tt=tt, vc=vc, wvv=wvv):
                    pv = bps.tile([128, 512], F32, tag="pq", name="pv")
                    for ci in range(CI):
                        nc.tensor.matmul(
                            pv[:, 0:256],
                            xT[ci][:, ts(tt, 128)],
                            wvv[:, ci, :],
                            start=(ci == 0), stop=(ci == CI - 1),
                        )
                    nc.vector.tensor_copy(
                        vv[:, tt, 4 * vc : 4 * vc + 4, 0:DH],
                        pv[:, 0:256].rearrange("p (h d) -> p h d", d=DH),
                    )
                yield step

        def att_steps(p, tqcs=(0, 1)):
            qkt = qktiles[p]
            for tqc in tqcs:
                ys = [
                    yps.tile([DH + 1, 512], F32, tag="ys", name="ys")
                    for _ in range(2)
                ]
                pend = {}

                dbg = dump and p == 0 and tqc == 0

                def scores_exp(tkt, tqc=tqc, qkt=qkt, pend=pend, dbg=dbg):
                    sp = sps.tile([128, 1024], F32, tag="sp", name="sp")
                    for sub in range(2):
                        po = sub * DH
                        nc.tensor.matmul(
                            sp[:, ts(sub, 512)],
                            qkt[po : po + DH, 1, ts(tkt, 128)],
                            qkt[po : po + DH, 0, ts(tqc, 512)],
                            start=True, stop=True,
                        )
                    e = esp.tile([128, 1024], BF16, tag="e", name="e")
                    nc.scalar.activation(e[:], sp[:], EXP, scale=SM_SCALE)
                    if dbg and tkt == 0:
                        nc.sync.dma_start(out=des_d[:], in_=e[:])
                    pend[tkt] = e

                def pv(tkt, ys=ys, p=p, pend=pend):
                    e = pend.pop(tkt)
                    for sub in range(2):
                        h = 2 * p + sub
                        nc.tensor.matmul(
                            ys[sub][:], vv[:, tkt, h, :], e[:, ts(sub, 512)],
                            start=(tkt == 0), stop=(tkt == TT - 1),
                        )

                # software pipeline: PV trails scores/exp by 3 tiles so the
                # exp->PV semaphore handoff stays off the critical path
                for tkt in range(TT):
                    def step(tkt=tkt, se=scores_exp, pvf=pv):
                        # PV first: its deps are long-met, so it runs even
                        # when the next scores matmul would head-of-line
                        # block on a PSUM buffer still being read by exp
                        if tkt > 2:
                            pvf(tkt - 3)
                        se(tkt)
                    yield step

                stgs = []
                ds = []

                def flush0(pvf=pv):
                    pvf(TT - 3)
                    pvf(TT - 2)
                yield flush0

                def flush(pvf=pv, ys=ys, stgs=stgs, ds=ds):
                    pvf(TT - 1)
                    # stage PSUM out immediately so the ys banks free up
                    for sub in range(2):
                        stg = nrmp.tile([DH + 1, 512], F32, tag="stg",
                                        name="stg", bufs=4)
                        nc.vector.tensor_copy(stg[:], ys[sub][:])
                        stgs.append(stg)
                        # denominator to partition 0: reciprocal_approx_fast
                        # mis-executes on inputs at a partition offset
                        d0 = nrmp.tile([1, 512], F32, tag="d0", name="d0",
                                       bufs=4)
                        nc.vector.tensor_copy(d0[:], ys[sub][DH : DH + 1, :])
                        ds.append(d0)
                yield flush

                def norm(tqc=tqc, p=p, dbg=dbg, stgs=stgs, ds=ds):
                    rs = []
                    for sub in range(2):
                        r = nrmp.tile([1, 512], F32, tag="r", name="r", bufs=4)
                        nc.vector.reciprocal_approx_fast(r[:], ds[sub][:])
                        rs.append(r)
                    rbs = []
                    for sub in range(2):
                        rb = nrmp.tile([DH, 512], F32, tag="rb", name="rb",
                                       bufs=4)
                        nc.gpsimd.partition_broadcast(rb[:], rs[sub][:])
                        rbs.append(rb)
                    if dbg:
                        nc.sync.dma_start(out=dstg_d[:], in_=stgs[0][:])
                        nc.sync.dma_start(out=drb_d[:], in_=rbs[0][:])
                    with nc.allow_low_precision(reason="bf16 attn output"):
                        for sub in range(2):
                            nc.vector.tensor_mul(
                                yt[ts(sub, DH), p, ts(tqc, 512)],
                                stgs[sub][0:DH, :], rbs[sub][:],
                            )
                yield norm

        def proj_steps(tts=range(TT), act_copy=False):
            for tt in tts:
                pos = [None]

                def step_a(tt=tt, pos=pos):
                    po = sps.tile([128, 1024], F32, tag="sp", name="po")
                    pos[0] = po
                    for ci in range(CI // 2):
                        for cch in range(2):
                            nc.tensor.matmul(
                                po[:, ts(cch, 512)],
                                yt[:, ci, ts(tt, 128)],
                                wpa[:, ci, ts(cch, 512)],
                                start=(ci == 0), stop=False,
                            )
                yield step_a

                def step_b(tt=tt, pos=pos):
                    po = pos[0]
                    for ci in range(CI // 2, CI):
                        for cch in range(2):
                            nc.tensor.matmul(
                                po[:, ts(cch, 512)],
                                yt[:, ci, ts(tt, 128)],
                                wpa[:, ci, ts(cch, 512)],
                                start=False, stop=(ci == CI - 1),
                            )
                    ot = otp.tile([128, C], F32, tag="ot", name="ot")
                    if act_copy:
                        nc.scalar.copy(ot[:], po[:])
                    else:
                        nc.vector.tensor_copy(ot[:], po[:])
                    nc.sync.dma_start(out=out_d[ts(tt, 128), :], in_=ot[:])
                yield step_b

        def run_all(gen):
            for s in gen:
                s()

        def zip_paced(builds, atts, lead=True):
            builds = list(builds)
            atts = list(atts)
            nb, na = len(builds), len(atts)
            bi = 0
            for ai, a in enumerate(atts):
                # lead: builds slightly ahead of att steps (data producers);
                # not lead: builds lag (fillers that depend on att progress)
                want = -(-nb * (ai + 1) // na) if lead else (nb * ai) // na
                while bi < want:
                    builds[bi]()
                    bi += 1
                a()
            while bi < nb:
                builds[bi]()
                bi += 1

        # front: q/k slab for pair 0 DMAs first (first matmuls need it),
        # then x slabs, then the v chunk 0 slab.
        qk0 = qk_build(0)
        v0 = v_build(0)
        next(qk0)()   # wqt0 DMA (sync queue)
        for ci in range(CI):
            nc.sync.dma_start(out=xT[ci][:], in_=xTp_d[ci])
        next(v0)()    # wvv0 DMA

        # Junk matmuls (short, never read) fill the initial DMA wait so the
        # PE's activity monitor un-throttles before real work arrives.
        nc.vector.memset(wrm[:, 0:128], 0.0)
        jnk = bps.tile([128, 512], F32, tag="pq", name="jnk")
        for i in range(32):
            nc.tensor.matmul(
                jnk[:, 0:128], wrm[:, 0:128], wrm[:, 0:128],
                start=(i == 0), stop=(i == 31),
            )
        nc.vector.memset(vv[:, :, :, DH : DH + 1], 1.0)

        # build q/k for pair 0 and v heads 0-3; prefetch wp
        run_all(chain(qk0, v0))
        if dump:
            nc.sync.dma_start(out=dqkt_d[:], in_=qktiles[0][:])
        nc.sync.dma_start(out=wpa[:], in_=wp_d[:])

        # steady state: attention for pair p zipped with builds for p+1
        zips = {
            0: chain(qk_build(1), v_build(1)),
            1: chain(qk_build(2), v_build(2)),
            2: chain(qk_build(3), v_build(3)),
            3: qk_build(4),
            4: qk_build(5),
            5: qk_build(6),
            6: qk_build(7),
        }
        for p in range(NPAIR - 1):
            zip_paced(zips[p], att_steps(p))

        # last pair: overlap the first half of the output projection with
        # the second tq chunk of its attention
        run_all(att_steps(NPAIR - 1, tqcs=(0,)))
        zip_paced(proj_steps(range(0, 4)), att_steps(NPAIR - 1, tqcs=(1,)))
        run_all(proj_steps(range(4, TT), act_copy=True))
        if dump:
            nc.sync.dma_start(out=dvv_d[:], in_=vv[:])
            nc.sync.dma_start(out=dyt_d[:], in_=yt[:])

    nc.compile()
    return nc


def make_in_maps(inputs):
    x = np.asarray(inputs["x"], dtype=np.float32)
    sid = np.asarray(inputs["subject_id"]).astype(np.int64)
    W_qkv = np.asarray(inputs["W_qkv"], dtype=np.float32)
    b_qkv = np.asarray(inputs["b_qkv"], dtype=np.float32)
    A1 = np.asarray(inputs["A1"], dtype=np.float32)
    B1 = np.asarray(inputs["B1"], dtype=np.float32)
    W_p = np.asarray(inputs["W_p"], dtype=np.float32)
    b_p = np.asarray(inputs["b_p"], dtype=np.float32)
    A2 = np.asarray(inputs["A2"], dtype=np.float32)
    B2 = np.asarray(inputs["B2"], dtype=np.float32)

    # The reference constructs zero biases; the kernel has no bias path.
    assert not np.any(b_qkv) and not np.any(b_p), "nonzero bias unsupported"

    wqkvT = np.ascontiguousarray(W_qkv.T)  # [C, 3C] fp32
    wpT = np.ascontiguousarray(W_p.T)      # [C, C] fp32

    def pack_slab(wT_slab):
        # [C, cols] -> [128, CI, cols] contiguous, bf16
        cols = wT_slab.shape[1]
        return np.ascontiguousarray(
            wT_slab.reshape(CI, 128, cols).transpose(1, 0, 2)
        ).astype(BF16NP)

    in_maps = []
    for b in range(NCORES):
        s = int(sid[b])
        w1T = wqkvT + AOR * (A1[s].T @ B1[s].T)   # [C, 3C]
        wpTe = wpT + AOR * (A2[s].T @ B2[s].T)    # [C, C]

        wqk = np.stack(
            [
                pack_slab(
                    np.concatenate(
                        [
                            w1T[:, p * 128 : (p + 1) * 128],
                            w1T[:, C + p * 128 : C + (p + 1) * 128],
                        ],
                        axis=1,
                    )
                )
                for p in range(NPAIR)
            ]
        )
        wv = np.stack(
            [
                pack_slab(w1T[:, 2 * C + vc * 256 : 2 * C + (vc + 1) * 256])
                for vc in range(NVC)
            ]
        )
        xTp = np.ascontiguousarray(x[b].T.reshape(CI, 128, T)).astype(BF16NP)

        in_maps.append(
            {
                "xTp": xTp,
                "wqk": np.ascontiguousarray(wqk),
                "wv": np.ascontiguousarray(wv),
                "wp": pack_slab(wpTe),
            }
        )
    return in_maps


_NC_CACHE = {}


def kernel(**inputs):
    if "nc" not in _NC_CACHE:
        _NC_CACHE["nc"] = _build()
    nc = _NC_CACHE["nc"]

    in_maps = make_in_maps(inputs)
    res = run_bass_kernel_spmd(nc, in_maps, core_ids=list(range(NCORES)))
    out = np.stack([r["out"] for r in res.results], axis=0)
    return out.astype(np.float32)


# revision 38
# speedup vs baseline: 1.0046x; 1.0046x over previous
"""LoRA-MHSA Trainium2 kernel, v2.

Data-parallel over batch B=8 (one sample per NeuronCore). The per-sample
LoRA adapter is folded into the base weights on the host (exact math:
W_eff = W + (alpha/rank) * B[sid] @ A[sid]), so the device kernel is a
plain MHSA over per-core effective weights.

All matmul operands are bf16 (fp32r streams at ~half rate on real HW;
bf16 streams a column per cycle and enables fast weight load). PSUM
accumulation stays fp32. Tolerance is 2e-2; bf16 lands ~1e-3.

Layout: activations channel-major ([C, T]) so q/k head slabs feed the
scores matmul directly. v is built in natural layout [T, C] with a
65th ones-column per head so the PV matmul emits the softmax denominator
for free in PSUM row 64. Scores for the two heads of a pair occupy
partitions 0-63 / 64-127, so their matmuls row-pack into disjoint PE
row-groups and run concurrently. One exp activation per (pair, tq-chunk,
tk-tile) covers both heads ([128, 1024] across 2 PSUM banks), halving
ACT instruction overhead. Softmax normalization is staged out of PSUM
immediately (frees the PV bank), then reciprocal_approx_fast + gpsimd
partition-broadcast + DVE multiply produce the normalized, bf16,
channel-major attention output off the critical path.

Weights are host-packed into per-slab contiguous blocks so every DMA is
a single contiguous transfer.
"""

import sys
from itertools import chain

sys.path.insert(0, "/opt/trn_rl_repo")

import numpy as np
import ml_dtypes
import concourse.bass as bass
import concourse.tile as tile
from concourse import bacc, mybir
from concourse.bass_utils import run_bass_kernel_spmd

T = 1024
C = 1024
C3 = 3072
H = 16
DH = 64
RANK = 8
AOR = 0.125          # alpha / rank
SM_SCALE = 0.125     # 1/sqrt(dh)
NCORES = 8

F32 = mybir.dt.float32
BF16 = mybir.dt.bfloat16
EXP = mybir.ActivationFunctionType.Exp
BF16NP = ml_dtypes.bfloat16

ts = bass.ts

TT = T // 128     # 8 tk tiles
TCH = T // 512    # 2 tq chunks (psum free dim)
CI = C // 128     # 8 contraction tiles
NPAIR = H // 2    # 8 head pairs
NVC = 4           # v built in 4 chunks of 256 couts (4 heads each)


def _build(dump=False):
    nc = bacc.Bacc("TRN2", target_bir_lowering=False, debug=False)

    # Host-packed layouts (see make_in_maps): every slab is contiguous.
    xTp_d = nc.dram_tensor("xTp", [CI, 128, T], BF16, kind="ExternalInput")
    wqk_d = nc.dram_tensor("wqk", [NPAIR, 128, CI, 256], BF16, kind="ExternalInput")
    wv_d = nc.dram_tensor("wv", [NVC, 128, CI, 256], BF16, kind="ExternalInput")
    wp_d = nc.dram_tensor("wp", [128, CI, C], BF16, kind="ExternalInput")
    out_d = nc.dram_tensor("out", [T, C], F32, kind="ExternalOutput")
    if dump:
        dqkt_d = nc.dram_tensor("dqkt", [128, 2, T], BF16, kind="ExternalOutput")
        dvv_d = nc.dram_tensor("dvv", [128, TT, H, DH + 1], BF16,
                               kind="ExternalOutput")
        des_d = nc.dram_tensor("des", [128, 1024], BF16, kind="ExternalOutput")
        dstg_d = nc.dram_tensor("dstg", [DH + 1, 512], F32, kind="ExternalOutput")
        drb_d = nc.dram_tensor("drb", [DH, 512], F32, kind="ExternalOutput")
        dyt_d = nc.dram_tensor("dyt", [128, CI, T], BF16, kind="ExternalOutput")

    with tile.TileContext(nc) as tc:
      with tc.tile_pool(name="res", bufs=1) as res, \
           tc.tile_pool(name="qkt", bufs=4) as qktp, \
           tc.tile_pool(name="wqk", bufs=2) as wqkp, \
           tc.tile_pool(name="wv", bufs=2) as wvp, \
           tc.tile_pool(name="es", bufs=5) as esp, \
           tc.tile_pool(name="nrm", bufs=2) as nrmp, \
           tc.tile_pool(name="ot", bufs=2) as otp, \
           tc.tile_pool(name="bps", bufs=2, space="PSUM") as bps, \
           tc.tile_pool(name="sps", bufs=2, space="PSUM") as sps, \
           tc.tile_pool(name="yps", bufs=2, space="PSUM") as yps:

        xT = [
            res.tile([128, T], BF16, tag=f"x{ci}", name=f"xT{ci}")
            for ci in range(CI)
        ]
        vv = res.tile([128, TT, H, DH + 1], BF16, tag="vv")
        yt = res.tile([128, CI, T], BF16, tag="yt")
        wpa = res.tile([128, CI, C], BF16, tag="wpa")
        wrm = res.tile([128, 512], BF16, tag="wrm")

        qktiles = {}

        def qk_build(p):
            wqt = wqkp.tile([128, CI, 256], BF16, tag="wq", name="wqt")

            def load(p=p, wqt=wqt):
                if p == 0:
                    # split so the first build matmuls (ci 0-1) unblock early
                    nc.sync.dma_start(out=wqt[:, 0:2, :], in_=wqk_d[p][:, 0:2, :])
                    nc.sync.dma_start(out=wqt[:, 2:CI, :], in_=wqk_d[p][:, 2:CI, :])
                else:
                    nc.sync.dma_start(out=wqt[:], in_=wqk_d[p])

            yield load
            qkt = qktp.tile([128, 2, T], BF16, tag="qkt", name="qkt")
            qktiles[p] = qkt
            for part in range(2):  # 0: q, 1: k
                pqs = [
                    bps.tile([128, 512], F32, tag="pq", name="pq")
                    for _ in range(TCH)
                ]
                for ci in range(CI):
                    def step(ci=ci, part=part, pqs=pqs, wqt=wqt):
                        for tch in range(TCH):
                            nc.tensor.matmul(
                                pqs[tch][:],
                                wqt[:, ci, ts(part, 128)],
                                xT[ci][:, ts(tch, 512)],
                                start=(ci == 0), stop=(ci == CI - 1),
                            )
                    yield step

                def fin(part=part, pqs=pqs, qkt=qkt):
                    for tch in range(TCH):
                        nc.vector.tensor_copy(
                            qkt[:, part, ts(tch, 512)], pqs[tch][:]
                        )
                yield fin

        def v_build(vc):  # heads 4*vc .. 4*vc+3
            wvv = wvp.tile([128, CI, 256], BF16, tag="wv", name="wvv")

            def load(vc=vc, wvv=wvv):
                nc.sync.dma_start(out=wvv[:], in_=wv_d[vc])

            yield load
            for tt in range(TT):
                def step(tt=tt, vc=vc, wvv=wvv):
                    pv = bps.tile([128, 512], F32, tag="pq", name="pv")
                    for ci in range(CI):
                        nc.tensor.matmul(
                            pv[:, 0:256],
                            xT[ci][:, ts(tt, 128)],
                            wvv[:, ci, :],
                            start=(ci == 0), stop=(ci == CI - 1),
                        )
                    nc.vector.tensor_copy(
                        vv[:, tt, 4 * vc : 4 * vc + 4, 0:DH],
                        pv[:, 0:256].rearrange("p (h d) -> p h d", d=DH),
                    )
                yield step

        def att_steps(p, tqcs=(0, 1), last=False):
            qkt = qktiles[p]
            for tqc in tqcs:
                ys = [
                    yps.tile([DH + 1, 512], F32, tag="ys", name="ys")
                    for _ in range(2)
                ]
                pend = {}

                dbg = dump and p == 0 and tqc == 0

                def scores_exp(tkt, tqc=tqc, qkt=qkt, pend=pend, dbg=dbg):
                    sp = sps.tile([128, 1024], F32, tag="sp", name="sp")
                    for sub in range(2):
                        po = sub * DH
                        nc.tensor.matmul(
                            sp[:, ts(sub, 512)],
                            qkt[po : po + DH, 1, ts(tkt, 128)],
                            qkt[po : po + DH, 0, ts(tqc, 512)],
                            start=True, stop=True,
                        )
                    e = esp.tile([128, 1024], BF16, tag="e", name="e")
                    nc.scalar.activation(e[:], sp[:], EXP, scale=SM_SCALE)
                    if dbg and tkt == 0:
                        nc.sync.dma_start(out=des_d[:], in_=e[:])
                    pend[tkt] = e

                def pv(tkt, ys=ys, p=p, pend=pend):
                    e = pend.pop(tkt)
                    for sub in range(2):
                        h = 2 * p + sub
                        nc.tensor.matmul(
                            ys[sub][:], vv[:, tkt, h, :], e[:, ts(sub, 512)],
                            start=(tkt == 0), stop=(tkt == TT - 1),
                        )

                # software pipeline: PV trails scores/exp by 3 tiles so the
                # exp->PV semaphore handoff stays off the critical path
                for tkt in range(TT):
                    def step(tkt=tkt, se=scores_exp, pvf=pv):
                        # PV first: its deps are long-met, so it runs even
                        # when the next scores matmul would head-of-line
                        # block on a PSUM buffer still being read by exp
                        if tkt > 2:
                            pvf(tkt - 3)
                        se(tkt)
                    yield step

                stgs = []
                ds = []

                def flush0(pvf=pv):
                    pvf(TT - 3)
                    pvf(TT - 2)
                yield flush0

                def flush(pvf=pv, ys=ys, stgs=stgs, ds=ds, last=last):
                    pvf(TT - 1)
                    # stage PSUM out immediately so the ys banks free up;
                    # on the final chunk nothing reuses the banks, so skip
                    # the staging copy and normalize from PSUM directly
                    for sub in range(2):
                        if not last:
                            stg = nrmp.tile([DH + 1, 512], F32, tag="stg",
                                            name="stg", bufs=4)
                            nc.vector.tensor_copy(stg[:], ys[sub][:])
                            stgs.append(stg)
                        else:
                            stgs.append(ys[sub])
                        # denominator to partition 0: reciprocal_approx_fast
                        # mis-executes on inputs at a partition offset
                        d0 = nrmp.tile([1, 512], F32, tag="d0", name="d0",
                                       bufs=4)
                        nc.vector.tensor_copy(d0[:], ys[sub][DH : DH + 1, :])
                        ds.append(d0)
                yield flush

                def norm(tqc=tqc, p=p, dbg=dbg, stgs=stgs, ds=ds):
                    rs = []
                    for sub in range(2):
                        r = nrmp.tile([1, 512], F32, tag="r", name="r", bufs=4)
                        nc.vector.reciprocal_approx_fast(r[:], ds[sub][:])
                        rs.append(r)
                    rbs = []
                    for sub in range(2):
                        rb = nrmp.tile([DH, 512], F32, tag="rb", name="rb",
                                       bufs=4)
                        nc.gpsimd.partition_broadcast(rb[:], rs[sub][:])
                        rbs.append(rb)
                    if dbg:
                        nc.sync.dma_start(out=dstg_d[:], in_=stgs[0][:])
                        nc.sync.dma_start(out=drb_d[:], in_=rbs[0][:])
                    with nc.allow_low_precision(reason="bf16 attn output"):
                        for sub in range(2):
                            nc.vector.tensor_mul(
                                yt[ts(sub, DH), p, ts(tqc, 512)],
                                stgs[sub][0:DH, :], rbs[sub][:],
                            )
                yield norm

        def proj_steps(tts=range(TT), act_copy=False):
            for tt in tts:
                pos = [None]

                def step_a(tt=tt, pos=pos):
                    po = sps.tile([128, 1024], F32, tag="sp", name="po")
                    pos[0] = po
                    for ci in range(CI // 2):
                        for cch in range(2):
                            nc.tensor.matmul(
                                po[:, ts(cch, 512)],
                                yt[:, ci, ts(tt, 128)],
                                wpa[:, ci, ts(cch, 512)],
                                start=(ci == 0), stop=False,
                            )
                yield step_a

                def step_b(tt=tt, pos=pos):
                    po = pos[0]
                    for ci in range(CI // 2, CI):
                        for cch in range(2):
                            nc.tensor.matmul(
                                po[:, ts(cch, 512)],
                                yt[:, ci, ts(tt, 128)],
                                wpa[:, ci, ts(cch, 512)],
                                start=False, stop=(ci == CI - 1),
                            )
                    ot = otp.tile([128, C], F32, tag="ot", name="ot")
                    if act_copy:
                        nc.scalar.copy(ot[:], po[:])
                    else:
                        nc.vector.tensor_copy(ot[:], po[:])
                    nc.sync.dma_start(out=out_d[ts(tt, 128), :], in_=ot[:])
                yield step_b

        def run_all(gen):
            for s in gen:
                s()

        def zip_paced(builds, atts, lead=True):
            builds = list(builds)
            atts = list(atts)
            nb, na = len(builds), len(atts)
            bi = 0
            for ai, a in enumerate(atts):
                # lead: builds slightly ahead of att steps (data producers);
                # not lead: builds lag (fillers that depend on att progress)
                want = -(-nb * (ai + 1) // na) if lead else (nb * ai) // na
                while bi < want:
                    builds[bi]()
                    bi += 1
                a()
            while bi < nb:
                builds[bi]()
                bi += 1

        # front: q/k slab for pair 0 DMAs first (first matmuls need it),
        # then x slabs, then the v chunk 0 slab.
        qk0 = qk_build(0)
        v0 = v_build(0)
        next(qk0)()   # wqt0 DMA (sync queue)
        for ci in range(CI):
            nc.sync.dma_start(out=xT[ci][:], in_=xTp_d[ci])
        next(v0)()    # wvv0 DMA

        # Junk matmuls (short, never read) fill the initial DMA wait so the
        # PE's activity monitor un-throttles before real work arrives.
        nc.vector.memset(wrm[:, 0:128], 0.0)
        jnk = bps.tile([128, 512], F32, tag="pq", name="jnk")
        for i in range(32):
            nc.tensor.matmul(
                jnk[:, 0:128], wrm[:, 0:128], wrm[:, 0:128],
                start=(i == 0), stop=(i == 31),
            )
        nc.vector.memset(vv[:, :, :, DH : DH + 1], 1.0)

        # build q/k for pair 0 and v heads 0-3; prefetch wp
        run_all(chain(qk0, v0))
        if dump:
            nc.sync.dma_start(out=dqkt_d[:], in_=qktiles[0][:])
        nc.sync.dma_start(out=wpa[:], in_=wp_d[:])

        # steady state: attention for pair p zipped with builds for p+1
        zips = {
            0: chain(qk_build(1), v_build(1)),
            1: chain(qk_build(2), v_build(2)),
            2: chain(qk_build(3), v_build(3)),
            3: qk_build(4),
            4: qk_build(5),
            5: qk_build(6),
            6: qk_build(7),
        }
        for p in range(NPAIR - 1):
            zip_paced(zips[p], att_steps(p))

        # last pair: overlap the first half of the output projection with
        # the second tq chunk of its attention
        run_all(att_steps(NPAIR - 1, tqcs=(0,)))
        zip_paced(proj_steps(range(0, 4)), att_steps(NPAIR - 1, tqcs=(1,), last=True))
        run_all(proj_steps(range(4, TT - 1), act_copy=True))
        # final tile: per-512-col copy+DMA so the store overlaps the
        # closing matmuls
        tt = TT - 1
        po = sps.tile([128, 1024], F32, tag="sp", name="pol")
        otl = otp.tile([128, C], F32, tag="ot", name="otl")
        for cch in range(2):
            for ci in range(CI):
                nc.tensor.matmul(
                    po[:, ts(cch, 512)],
                    yt[:, ci, ts(tt, 128)],
                    wpa[:, ci, ts(cch, 512)],
                    start=(ci == 0), stop=(ci == CI - 1),
                )
            nc.scalar.copy(otl[:, ts(cch, 512)], po[:, ts(cch, 512)])
            nc.sync.dma_start(
                out=out_d[ts(tt, 128), cch * 512 : cch * 512 + 512],
                in_=otl[:, ts(cch, 512)],
            )
        if dump:
            nc.sync.dma_start(out=dvv_d[:], in_=vv[:])
            nc.sync.dma_start(out=dyt_d[:], in_=yt[:])

    nc.compile()
    return nc


def make_in_maps(inputs):
    x = np.asarray(inputs["x"], dtype=np.float32)
    sid = np.asarray(inputs["subject_id"]).astype(np.int64)
    W_qkv = np.asarray(inputs["W_qkv"], dtype=np.float32)
    b_qkv = np.asarray(inputs["b_qkv"], dtype=np.float32)
    A1 = np.asarray(inputs["A1"], dtype=np.float32)
    B1 = np.asarray(inputs["B1"], dtype=np.float32)
    W_p = np.asarray(inputs["W_p"], dtype=np.float32)
    b_p = np.asarray(inputs["b_p"], dtype=np.float32)
    A2 = np.asarray(inputs["A2"], dtype=np.float32)
    B2 = np.asarray(inputs["B2"], dtype=np.float32)

    # The reference constructs zero biases; the kernel has no bias path.
    assert not np.any(b_qkv) and not np.any(b_p), "nonzero bias unsupported"

    wqkvT = np.ascontiguousarray(W_qkv.T)  # [C, 3C] fp32
    wpT = np.ascontiguousarray(W_p.T)      # [C, C] fp32

    def pack_slab(wT_slab):
        # [C, cols] -> [128, CI, cols] contiguous, bf16
        cols = wT_slab.shape[1]
        return np.ascontiguousarray(
            wT_slab.reshape(CI, 128, cols).transpose(1, 0, 2)
        ).astype(BF16NP)

    in_maps = []
    for b in range(NCORES):
        s = int(sid[b])
        w1T = wqkvT + AOR * (A1[s].T @ B1[s].T)   # [C, 3C]
        wpTe = wpT + AOR * (A2[s].T @ B2[s].T)    # [C, C]

        wqk = np.stack(
            [
                pack_slab(
                    np.concatenate(
                        [
                            w1T[:, p * 128 : (p + 1) * 128],
                            w1T[:, C + p * 128 : C + (p + 1) * 128],
                        ],
                        axis=1,
                    )
                )
                for p in range(NPAIR)
            ]
        )
        wv = np.stack(
            [
                pack_slab(w1T[:, 2 * C + vc * 256 : 2 * C + (vc + 1) * 256])
                for vc in range(NVC)
            ]
        )
        xTp = np.ascontiguousarray(x[b].T.reshape(CI, 128, T)).astype(BF16NP)

        in_maps.append(
            {
                "xTp": xTp,
                "wqk": np.ascontiguousarray(wqk),
                "wv": np.ascontiguousarray(wv),
                "wp": pack_slab(wpTe),
            }
        )
    return in_maps


_NC_CACHE = {}


def kernel(**inputs):
    if "nc" not in _NC_CACHE:
        _NC_CACHE["nc"] = _build()
    nc = _NC_CACHE["nc"]

    in_maps = make_in_maps(inputs)
    res = run_bass_kernel_spmd(nc, in_maps, core_ids=list(range(NCORES)))
    out = np.stack([r["out"] for r in res.results], axis=0)
    return out.astype(np.float32)


# revision 40
# speedup vs baseline: 1.0186x; 1.0140x over previous
"""LoRA-MHSA Trainium2 kernel, v2.

Data-parallel over batch B=8 (one sample per NeuronCore). The per-sample
LoRA adapter is folded into the base weights on the host (exact math:
W_eff = W + (alpha/rank) * B[sid] @ A[sid]), so the device kernel is a
plain MHSA over per-core effective weights.

All matmul operands are bf16 (fp32r streams at ~half rate on real HW;
bf16 streams a column per cycle and enables fast weight load). PSUM
accumulation stays fp32. Tolerance is 2e-2; bf16 lands ~1e-3.

Layout: activations channel-major ([C, T]) so q/k head slabs feed the
scores matmul directly. v is built in natural layout [T, C] with a
65th ones-column per head so the PV matmul emits the softmax denominator
for free in PSUM row 64. Scores for the two heads of a pair occupy
partitions 0-63 / 64-127, so their matmuls row-pack into disjoint PE
row-groups and run concurrently. One exp activation per (pair, tq-chunk,
tk-tile) covers both heads ([128, 1024] across 2 PSUM banks), halving
ACT instruction overhead. Softmax normalization is staged out of PSUM
immediately (frees the PV bank), then reciprocal_approx_fast + gpsimd
partition-broadcast + DVE multiply produce the normalized, bf16,
channel-major attention output off the critical path.

Weights are host-packed into per-slab contiguous blocks so every DMA is
a single contiguous transfer.
"""

import sys
from itertools import chain

sys.path.insert(0, "/opt/trn_rl_repo")

import numpy as np
import ml_dtypes
import concourse.bass as bass
import concourse.tile as tile
from concourse import bacc, mybir
from concourse.bass_utils import run_bass_kernel_spmd

T = 1024
C = 1024
C3 = 3072
H = 16
DH = 64
RANK = 8
AOR = 0.125          # alpha / rank
SM_SCALE = 0.125     # 1/sqrt(dh)
NCORES = 8

F32 = mybir.dt.float32
BF16 = mybir.dt.bfloat16
EXP = mybir.ActivationFunctionType.Exp
BF16NP = ml_dtypes.bfloat16

ts = bass.ts

TT = T // 128     # 8 tk tiles
TCH = T // 512    # 2 tq chunks (psum free dim)
CI = C // 128     # 8 contraction tiles
NPAIR = H // 2    # 8 head pairs
NVC = 4           # v built in 4 chunks of 256 couts (4 heads each)


def _build(dump=False):
    nc = bacc.Bacc("TRN2", target_bir_lowering=False, debug=False)

    # Host-packed layouts (see make_in_maps): every slab is contiguous.
    xTp_d = nc.dram_tensor("xTp", [CI, 128, T], BF16, kind="ExternalInput")
    wqk_d = nc.dram_tensor("wqk", [NPAIR, 128, CI, 256], BF16, kind="ExternalInput")
    wv_d = nc.dram_tensor("wv", [NVC, 128, CI, 256], BF16, kind="ExternalInput")
    wp_d = nc.dram_tensor("wp", [128, CI, C], BF16, kind="ExternalInput")
    out_d = nc.dram_tensor("out", [T, C], F32, kind="ExternalOutput")
    if dump:
        dqkt_d = nc.dram_tensor("dqkt", [128, 2, T], BF16, kind="ExternalOutput")
        dvv_d = nc.dram_tensor("dvv", [128, TT, H, DH + 1], BF16,
                               kind="ExternalOutput")
        des_d = nc.dram_tensor("des", [128, 1024], BF16, kind="ExternalOutput")
        dstg_d = nc.dram_tensor("dstg", [DH + 1, 512], F32, kind="ExternalOutput")
        drb_d = nc.dram_tensor("drb", [DH, 512], F32, kind="ExternalOutput")
        dyt_d = nc.dram_tensor("dyt", [128, CI, T], BF16, kind="ExternalOutput")

    with tile.TileContext(nc) as tc:
      with tc.tile_pool(name="res", bufs=1) as res, \
           tc.tile_pool(name="qkt", bufs=4) as qktp, \
           tc.tile_pool(name="wqk", bufs=2) as wqkp, \
           tc.tile_pool(name="wv", bufs=2) as wvp, \
           tc.tile_pool(name="es", bufs=5) as esp, \
           tc.tile_pool(name="nrm", bufs=2) as nrmp, \
           tc.tile_pool(name="ot", bufs=2) as otp, \
           tc.tile_pool(name="bps", bufs=2, space="PSUM") as bps, \
           tc.tile_pool(name="sps", bufs=2, space="PSUM") as sps, \
           tc.tile_pool(name="yps", bufs=2, space="PSUM") as yps:

        xT = [
            res.tile([128, T], BF16, tag=f"x{ci}", name=f"xT{ci}")
            for ci in range(CI)
        ]
        vv = res.tile([128, TT, H, DH + 1], BF16, tag="vv")
        yt = res.tile([128, CI, T], BF16, tag="yt")
        wpa = res.tile([128, CI, C], BF16, tag="wpa")
        wrm = res.tile([128, 512], BF16, tag="wrm")

        qktiles = {}

        def qk_build(p):
            wqt = wqkp.tile([128, CI, 256], BF16, tag="wq", name="wqt")

            def load(p=p, wqt=wqt):
                if p == 0:
                    # split so the first build matmuls (ci 0-1) unblock early
                    nc.sync.dma_start(out=wqt[:, 0:2, :], in_=wqk_d[p][:, 0:2, :])
                    nc.sync.dma_start(out=wqt[:, 2:CI, :], in_=wqk_d[p][:, 2:CI, :])
                else:
                    nc.sync.dma_start(out=wqt[:], in_=wqk_d[p])

            yield load
            qkt = qktp.tile([128, 2, T], BF16, tag="qkt", name="qkt")
            qktiles[p] = qkt
            for part in range(2):  # 0: q, 1: k
                pqs = [
                    bps.tile([128, 512], F32, tag="pq", name="pq")
                    for _ in range(TCH)
                ]
                for ci in range(CI):
                    def step(ci=ci, part=part, pqs=pqs, wqt=wqt):
                        for tch in range(TCH):
                            nc.tensor.matmul(
                                pqs[tch][:],
                                wqt[:, ci, ts(part, 128)],
                                xT[ci][:, ts(tch, 512)],
                                start=(ci == 0), stop=(ci == CI - 1),
                            )
                    yield step

                def fin(part=part, pqs=pqs, qkt=qkt):
                    for tch in range(TCH):
                        nc.vector.tensor_copy(
                            qkt[:, part, ts(tch, 512)], pqs[tch][:]
                        )
                yield fin

        def v_build(vc):  # heads 4*vc .. 4*vc+3
            wvv = wvp.tile([128, CI, 256], BF16, tag="wv", name="wvv")

            def load(vc=vc, wvv=wvv):
                nc.sync.dma_start(out=wvv[:], in_=wv_d[vc])

            yield load
            for tt in range(TT):
                def step(tt=tt, vc=vc, wvv=wvv):
                    pv = bps.tile([128, 512], F32, tag="pq", name="pv")
                    for ci in range(CI):
                        nc.tensor.matmul(
                            pv[:, 0:256],
                            xT[ci][:, ts(tt, 128)],
                            wvv[:, ci, :],
                            start=(ci == 0), stop=(ci == CI - 1),
                        )
                    nc.vector.tensor_copy(
                        vv[:, tt, 4 * vc : 4 * vc + 4, 0:DH],
                        pv[:, 0:256].rearrange("p (h d) -> p h d", d=DH),
                    )
                yield step

        def att_steps(p, tqcs=(0, 1), last=False):
            qkt = qktiles[p]
            for tqc in tqcs:
                ys = [
                    yps.tile([DH + 1, 512], F32, tag="ys", name="ys")
                    for _ in range(2)
                ]
                pend = {}

                dbg = dump and p == 0 and tqc == 0

                def scores_exp(tkt, tqc=tqc, qkt=qkt, pend=pend, dbg=dbg):
                    sp = sps.tile([128, 1024], F32, tag="sp", name="sp")
                    for sub in range(2):
                        po = sub * DH
                        nc.tensor.matmul(
                            sp[:, ts(sub, 512)],
                            qkt[po : po + DH, 1, ts(tkt, 128)],
                            qkt[po : po + DH, 0, ts(tqc, 512)],
                            start=True, stop=True,
                        )
                    e = esp.tile([128, 1024], BF16, tag="e", name="e")
                    nc.scalar.activation(e[:], sp[:], EXP, scale=SM_SCALE)
                    if dbg and tkt == 0:
                        nc.sync.dma_start(out=des_d[:], in_=e[:])
                    pend[tkt] = e

                def pv(tkt, ys=ys, p=p, pend=pend):
                    e = pend.pop(tkt)
                    for sub in range(2):
                        h = 2 * p + sub
                        nc.tensor.matmul(
                            ys[sub][:], vv[:, tkt, h, :], e[:, ts(sub, 512)],
                            start=(tkt == 0), stop=(tkt == TT - 1),
                        )

                # software pipeline: PV trails scores/exp by 3 tiles so the
                # exp->PV semaphore handoff stays off the critical path
                for tkt in range(TT):
                    def step(tkt=tkt, se=scores_exp, pvf=pv):
                        # PV first: its deps are long-met, so it runs even
                        # when the next scores matmul would head-of-line
                        # block on a PSUM buffer still being read by exp
                        if tkt > 2:
                            pvf(tkt - 3)
                        se(tkt)
                    yield step

                stgs = []
                ds = []

                def flush0(pvf=pv):
                    pvf(TT - 3)
                    pvf(TT - 2)
                yield flush0

                def flush(pvf=pv, ys=ys, stgs=stgs, ds=ds, last=last):
                    pvf(TT - 1)
                    # stage PSUM out immediately so the ys banks free up;
                    # on the final chunk nothing reuses the banks, so skip
                    # the staging copy and normalize from PSUM directly
                    for sub in range(2):
                        if not last:
                            stg = nrmp.tile([DH + 1, 512], F32, tag="stg",
                                            name="stg", bufs=4)
                            nc.vector.tensor_copy(stg[:], ys[sub][:])
                            stgs.append(stg)
                        else:
                            stgs.append(ys[sub])
                        # denominator to partition 0: reciprocal_approx_fast
                        # mis-executes on inputs at a partition offset
                        d0 = nrmp.tile([1, 512], F32, tag="d0", name="d0",
                                       bufs=4)
                        nc.vector.tensor_copy(d0[:], ys[sub][DH : DH + 1, :])
                        ds.append(d0)
                yield flush

                def norm(tqc=tqc, p=p, dbg=dbg, stgs=stgs, ds=ds):
                    rs = []
                    for sub in range(2):
                        r = nrmp.tile([1, 512], F32, tag="r", name="r", bufs=4)
                        nc.vector.reciprocal_approx_fast(r[:], ds[sub][:])
                        rs.append(r)
                    rbs = []
                    for sub in range(2):
                        rb = nrmp.tile([DH, 512], F32, tag="rb", name="rb",
                                       bufs=4)
                        nc.gpsimd.partition_broadcast(rb[:], rs[sub][:])
                        rbs.append(rb)
                    if dbg:
                        nc.sync.dma_start(out=dstg_d[:], in_=stgs[0][:])
                        nc.sync.dma_start(out=drb_d[:], in_=rbs[0][:])
                    with nc.allow_low_precision(reason="bf16 attn output"):
                        for sub in range(2):
                            nc.vector.tensor_mul(
                                yt[ts(sub, DH), p, ts(tqc, 512)],
                                stgs[sub][0:DH, :], rbs[sub][:],
                            )
                yield norm

        def proj_steps(tts=range(TT), act_copy=False):
            for tt in tts:
                pos = [None]

                def step_a(tt=tt, pos=pos):
                    po = sps.tile([128, 1024], F32, tag="sp", name="po")
                    pos[0] = po
                    for ci in range(CI // 2):
                        for cch in range(2):
                            nc.tensor.matmul(
                                po[:, ts(cch, 512)],
                                yt[:, ci, ts(tt, 128)],
                                wpa[:, ci, ts(cch, 512)],
                                start=(ci == 0), stop=False,
                            )
                yield step_a

                def step_b(tt=tt, pos=pos):
                    po = pos[0]
                    for ci in range(CI // 2, CI):
                        for cch in range(2):
                            nc.tensor.matmul(
                                po[:, ts(cch, 512)],
                                yt[:, ci, ts(tt, 128)],
                                wpa[:, ci, ts(cch, 512)],
                                start=False, stop=(ci == CI - 1),
                            )
                    ot = otp.tile([128, C], F32, tag="ot", name="ot")
                    if act_copy:
                        nc.scalar.copy(ot[:], po[:])
                    else:
                        nc.vector.tensor_copy(ot[:], po[:])
                    nc.sync.dma_start(out=out_d[ts(tt, 128), :], in_=ot[:])
                yield step_b

        def run_all(gen):
            for s in gen:
                s()

        def zip_paced(builds, atts, lead=True):
            builds = list(builds)
            atts = list(atts)
            nb, na = len(builds), len(atts)
            bi = 0
            for ai, a in enumerate(atts):
                # lead: builds slightly ahead of att steps (data producers);
                # not lead: builds lag (fillers that depend on att progress)
                want = -(-nb * (ai + 1) // na) if lead else (nb * ai) // na
                while bi < want:
                    builds[bi]()
                    bi += 1
                a()
            while bi < nb:
                builds[bi]()
                bi += 1

        # front: q/k slab for pair 0 DMAs first (first matmuls need it),
        # then x slabs, then the v chunk 0 slab.
        qk0 = qk_build(0)
        v0 = v_build(0)
        next(qk0)()   # wqt0 DMA (sync queue)
        for ci in range(CI):
            nc.sync.dma_start(out=xT[ci][:], in_=xTp_d[ci])
        next(v0)()    # wvv0 DMA

        # Junk matmuls (short, never read) fill the initial DMA wait so the
        # PE's activity monitor un-throttles before real work arrives.
        nc.vector.memset(wrm[:, 0:128], 0.0)
        jnk = bps.tile([128, 512], F32, tag="pq", name="jnk")
        for i in range(32):
            nc.tensor.matmul(
                jnk[:, 0:128], wrm[:, 0:128], wrm[:, 0:128],
                start=(i == 0), stop=(i == 31),
            )
        nc.vector.memset(vv[:, :, :, DH : DH + 1], 1.0)

        # build q/k for pair 0 and v heads 0-3; prefetch wp
        run_all(chain(qk0, v0))
        if dump:
            nc.sync.dma_start(out=dqkt_d[:], in_=qktiles[0][:])
        nc.sync.dma_start(out=wpa[:], in_=wp_d[:])

        # steady state: attention for pair p zipped with builds for p+1
        zips = {
            0: chain(qk_build(1), v_build(1)),
            1: chain(qk_build(2), v_build(2)),
            2: chain(qk_build(3), v_build(3)),
            3: qk_build(4),
            4: qk_build(5),
            5: qk_build(6),
            6: qk_build(7),
        }
        for p in range(NPAIR - 1):
            zip_paced(zips[p], att_steps(p))

        # last pair: the first output-projection tile only needs pair 7 in
        # its final accumulation step, so its pairs-0..6 partial fills the
        # otherwise build-less first tq chunk of the last attention pair
        pob = [
            bps.tile([128, 512], F32, tag="pq", name=f"pob{cch}")
            for cch in range(2)
        ]

        def proj0_partial():
            for a, b in ((0, 3), (3, 5), (5, 7)):
                def stepp(a=a, b=b):
                    for ci in range(a, b):
                        for cch in range(2):
                            nc.tensor.matmul(
                                pob[cch][:],
                                yt[:, ci, ts(0, 128)],
                                wpa[:, ci, ts(cch, 512)],
                                start=(ci == 0), stop=False,
                            )
                yield stepp

        def proj0_fin():
            ot = otp.tile([128, C], F32, tag="ot", name="ot0")
            for cch in range(2):
                nc.tensor.matmul(
                    pob[cch][:],
                    yt[:, CI - 1, ts(0, 128)],
                    wpa[:, CI - 1, ts(cch, 512)],
                    start=False, stop=True,
                )
                nc.vector.tensor_copy(ot[:, ts(cch, 512)], pob[cch][:])
            nc.sync.dma_start(out=out_d[ts(0, 128), :], in_=ot[:])

        zip_paced(proj0_partial(), att_steps(NPAIR - 1, tqcs=(0,)))
        zip_paced(chain([proj0_fin], proj_steps(range(1, 4))),
                  att_steps(NPAIR - 1, tqcs=(1,), last=True))
        run_all(proj_steps(range(4, TT - 1), act_copy=True))
        # final tile: per-512-col copy+DMA so the store overlaps the
        # closing matmuls
        tt = TT - 1
        po = sps.tile([128, 1024], F32, tag="sp", name="pol")
        otl = otp.tile([128, C], F32, tag="ot", name="otl")
        for cch in range(2):
            for ci in range(CI):
                nc.tensor.matmul(
                    po[:, ts(cch, 512)],
                    yt[:, ci, ts(tt, 128)],
                    wpa[:, ci, ts(cch, 512)],
                    start=(ci == 0), stop=(ci == CI - 1),
                )
            nc.scalar.copy(otl[:, ts(cch, 512)], po[:, ts(cch, 512)])
            nc.sync.dma_start(
                out=out_d[ts(tt, 128), cch * 512 : cch * 512 + 512],
                in_=otl[:, ts(cch, 512)],
            )
        if dump:
            nc.sync.dma_start(out=dvv_d[:], in_=vv[:])
            nc.sync.dma_start(out=dyt_d[:], in_=yt[:])

    nc.compile()
    return nc


def make_in_maps(inputs):
    x = np.asarray(inputs["x"], dtype=np.float32)
    sid = np.asarray(inputs["subject_id"]).astype(np.int64)
    W_qkv = np.asarray(inputs["W_qkv"], dtype=np.float32)
    b_qkv = np.asarray(inputs["b_qkv"], dtype=np.float32)
    A1 = np.asarray(inputs["A1"], dtype=np.float32)
    B1 = np.asarray(inputs["B1"], dtype=np.float32)
    W_p = np.asarray(inputs["W_p"], dtype=np.float32)
    b_p = np.asarray(inputs["b_p"], dtype=np.float32)
    A2 = np.asarray(inputs["A2"], dtype=np.float32)
    B2 = np.asarray(inputs["B2"], dtype=np.float32)

    # The reference constructs zero biases; the kernel has no bias path.
    assert not np.any(b_qkv) and not np.any(b_p), "nonzero bias unsupported"

    wqkvT = np.ascontiguousarray(W_qkv.T)  # [C, 3C] fp32
    wpT = np.ascontiguousarray(W_p.T)      # [C, C] fp32

    def pack_slab(wT_slab):
        # [C, cols] -> [128, CI, cols] contiguous, bf16
        cols = wT_slab.shape[1]
        return np.ascontiguousarray(
            wT_slab.reshape(CI, 128, cols).transpose(1, 0, 2)
        ).astype(BF16NP)

    in_maps = []
    for b in range(NCORES):
        s = int(sid[b])
        w1T = wqkvT + AOR * (A1[s].T @ B1[s].T)   # [C, 3C]
        wpTe = wpT + AOR * (A2[s].T @ B2[s].T)    # [C, C]

        wqk = np.stack(
            [
                pack_slab(
                    np.concatenate(
                        [
                            w1T[:, p * 128 : (p + 1) * 128],
                            w1T[:, C + p * 128 : C + (p + 1) * 128],
                        ],
                        axis=1,
                    )
                )
                for p in range(NPAIR)
            ]
        )
        wv = np.stack(
            [
                pack_slab(w1T[:, 2 * C + vc * 256 : 2 * C + (vc + 1) * 256])
                for vc in range(NVC)
            ]
        )
        xTp = np.ascontiguousarray(x[b].T.reshape(CI, 128, T)).astype(BF16NP)

        in_maps.append(
            {
                "xTp": xTp,
                "wqk": np.ascontiguousarray(wqk),
                "wv": np.ascontiguousarray(wv),
                "wp": pack_slab(wpTe),
            }
        )
    return in_maps


_NC_CACHE = {}


def kernel(**inputs):
    if "nc" not in _NC_CACHE:
        _NC_CACHE["nc"] = _build()
    nc = _NC_CACHE["nc"]

    in_maps = make_in_maps(inputs)
    res = run_bass_kernel_spmd(nc, in_maps, core_ids=list(range(NCORES)))
    out = np.stack([r["out"] for r in res.results], axis=0)
    return out.astype(np.float32)
